# revision 8
# baseline (speedup 1.0000x reference)
"""Trainium2 Bass kernel for nn_AdaptedEntropyModel (vq_codebook).

reference:
    r = x - means
    symbols = argmin_i |codebook[i] - r|   (ties -> left / lower index)
    y_hat   = codebook[symbols] + means

Algorithm used here (exact, including tie behavior):
    with midpoints m_i = (c_i + c_{i+1})/2, i = 0..L-2 (codebook sorted):
        symbols = sum_i [r > m_i]
        y_hat   = c_0 + sum_i [r > m_i] * (c_{i+1} - c_i) + means

Per-core the heavy loop is 63 fused scalar_tensor_tensor ops (compare +
accumulate) for symbols and 63 fused tensor_scalar (compare * delta) +
tensor_tensor add ops for the values, all on the DVE (plus a slice of
tiles offloaded to GPSIMD).

Sharding: pure data parallel. x/means are [32,192,64,64]; each of the 8
cores gets 4 consecutive batches (a contiguous 3,145,728-element chunk),
viewed as [128, 24576] on device. x and means are interleaved host-side
into one [2, 128, FREE] input so each tile is loaded by a single DMA
(one wait semaphore - the V3 ISA allows only one sync wait per compute
instruction). No communication. The tiny codebook is turned into
per-instruction immediates at build time (kernel() re-builds if it ever
sees a different codebook).
"""

import sys

import numpy as np

if "/opt/trn_rl_repo" not in sys.path:
    sys.path.insert(0, "/opt/trn_rl_repo")

B, C, H, W = 32, 192, 64, 64
L = 64
N_CORES = 8
TOT = B * C * H * W            # 25_165_824
PER_CORE = TOT // N_CORES      # 3_145_728
P = 128
FREE = PER_CORE // P           # 24576
TILE_F = 2048
N_TILES = FREE // TILE_F       # 12
# tiles handed to gpsimd instead of the vector engine (load balance)
GPSIMD_TILES = 0


def _build(mids, deltas, c0):
    """Build the per-core SPMD Bass program. mids/deltas are python float
    lists of length L-1; c0 = codebook[0]."""
    from contextlib import ExitStack

    import concourse.bass as bass
    import concourse.tile as tile
    from concourse import bacc, mybir

    f32 = mybir.dt.float32
    i32 = mybir.dt.int32
    Alu = mybir.AluOpType

    nc = bacc.Bacc(
        "TRN2",
        target_bir_lowering=False,
        debug=False,
        num_devices=N_CORES,
    )
    # row p = [x row | means row], so one DMA (one wait sem) per tile feeds
    # both operands with the partition dim leading in both APs.
    xm = nc.dram_tensor("xm", [P, 2 * FREE], f32, kind="ExternalInput")
    xm_r = xm.rearrange("p (h q) -> p h q", h=2)
    sym_out = nc.dram_tensor("sym", [P, FREE], i32, kind="ExternalOutput")
    y_out = nc.dram_tensor("y", [P, FREE], f32, kind="ExternalOutput")

    with tile.TileContext(nc) as tc, ExitStack() as ctx:
        inp = ctx.enter_context(tc.tile_pool(name="inp", bufs=3))
        work = ctx.enter_context(tc.tile_pool(name="work", bufs=1))
        outp = ctx.enter_context(tc.tile_pool(name="outp", bufs=2))

        for t in range(N_TILES):
            eng = nc.gpsimd if t < GPSIMD_TILES else nc.vector
            sl = bass.ts(t, TILE_F)
            # one DMA loads both x and means tile halves -> single wait sem
            txm = inp.tile([P, 2 * TILE_F], f32, tag="txm")
            nc.gpsimd.dma_start(
                txm[:].rearrange("p (h f) -> p h f", h=2), xm_r[:, :, sl]
            )
            tx = txm[:, :TILE_F]
            tm = txm[:, TILE_F:]

            r = work.tile([P, TILE_F], f32, tag=f"r{t % 2}")
            eng.tensor_sub(r[:], tx, tm)

            # symbols: s = sum_i (r > m_i), ping-pong accumulator
            sa = work.tile([P, TILE_F], f32, tag=f"sa{t % 2}")
            sb = work.tile([P, TILE_F], f32, tag=f"sb{t % 2}")
            eng.tensor_scalar(sa[:], r[:], mids[0], None, op0=Alu.is_gt)
            cur, nxt = sa, sb
            for i in range(1, L - 1):
                eng.scalar_tensor_tensor(
                    nxt[:], r[:], mids[i], cur[:], op0=Alu.is_gt, op1=Alu.add
                )
                cur, nxt = nxt, cur

            syi = outp.tile([P, TILE_F], i32, tag="syi")
            eng.tensor_copy(syi[:], cur[:])
            nc.sync.dma_start(sym_out[:, sl], syi[:])

            # values: y = sum_i (r > m_i) * delta_i   (+ c0 + means at the end)
            ya = work.tile([P, TILE_F], f32, tag=f"ya{t % 2}")
            yb = work.tile([P, TILE_F], f32, tag=f"yb{t % 2}")
            tmp = work.tile([P, TILE_F], f32, tag=f"tmp{t % 2}")
            eng.tensor_scalar(
                ya[:], r[:], mids[0], deltas[0], op0=Alu.is_gt, op1=Alu.mult
            )
            cur, nxt = ya, yb
            for i in range(1, L - 1):
                eng.tensor_scalar(
                    tmp[:], r[:], mids[i], deltas[i], op0=Alu.is_gt, op1=Alu.mult
                )
                eng.tensor_add(nxt[:], cur[:], tmp[:])
                cur, nxt = nxt, cur

            yh = outp.tile([P, TILE_F], f32, tag="yh")
            eng.scalar_tensor_tensor(
                yh[:], tm, c0, cur[:], op0=Alu.add, op1=Alu.add
            )
            nc.sync.dma_start(y_out[:, sl], yh[:])

    nc.compile()
    return nc


_cache = {}


def _get_nc(codebook):
    key = codebook.tobytes()
    if key not in _cache:
        cb = codebook.astype(np.float64)
        mids = [float(v) for v in ((cb[:-1] + cb[1:]) * 0.5).astype(np.float32)]
        deltas = [float(v) for v in (cb[1:] - cb[:-1]).astype(np.float32)]
        _cache[key] = _build(mids, deltas, float(codebook[0]))
    return _cache[key]


def _run(x, means, codebook, trace=False):
    from concourse.bass_utils import run_bass_kernel_spmd

    nc = _get_nc(np.asarray(codebook))

    x = np.asarray(x).reshape(N_CORES, P, FREE)
    means = np.asarray(means).reshape(N_CORES, P, FREE)
    in_maps = [
        {"xm": np.ascontiguousarray(np.concatenate([x[c], means[c]], axis=1))}
        for c in range(N_CORES)
    ]
    res = run_bass_kernel_spmd(
        nc, in_maps, core_ids=list(range(N_CORES)), trace=trace
    )
    sym = np.stack([res.results[c]["sym"] for c in range(N_CORES)])
    y = np.stack([res.results[c]["y"] for c in range(N_CORES)])
    sym = sym.reshape(B, C, H, W).astype(np.int32)
    y = y.reshape(B, C, H, W).astype(np.float32)
    return (sym, y), res


def kernel(x, means, codebook):
    (sym, y), _ = _run(x, means, codebook)
    return sym, y


# revision 13
# speedup vs baseline: 1.0994x; 1.0994x over previous
"""Trainium2 Bass kernel for nn_AdaptedEntropyModel (vq_codebook).

reference:
    r = x - means
    symbols = argmin_i |codebook[i] - r|   (ties -> left / lower index)
    y_hat   = codebook[symbols] + means

Algorithm used here (exact, including tie behavior):
    with midpoints m_i = (c_i + c_{i+1})/2, i = 0..L-2 (codebook sorted):
        symbols = sum_i [r > m_i]
        y_hat   = c_0 + sum_i [r > m_i] * (c_{i+1} - c_i) + means

Per-core the heavy loop is 63 fused scalar_tensor_tensor ops (compare +
accumulate) for symbols and 63 fused tensor_scalar (compare * delta) +
tensor_tensor add ops for the values, all on the DVE (plus a slice of
tiles offloaded to GPSIMD).

Sharding: pure data parallel. x/means are [32,192,64,64]; each of the 8
cores gets 4 consecutive batches (a contiguous 3,145,728-element chunk),
viewed as [128, 24576] on device. x and means are interleaved host-side
into one [2, 128, FREE] input so each tile is loaded by a single DMA
(one wait semaphore - the V3 ISA allows only one sync wait per compute
instruction). No communication. The tiny codebook is turned into
per-instruction immediates at build time (kernel() re-builds if it ever
sees a different codebook).
"""

import sys

import numpy as np

if "/opt/trn_rl_repo" not in sys.path:
    sys.path.insert(0, "/opt/trn_rl_repo")

B, C, H, W = 32, 192, 64, 64
L = 64
N_CORES = 8
TOT = B * C * H * W            # 25_165_824
PER_CORE = TOT // N_CORES      # 3_145_728
P = 128
FREE = PER_CORE // P           # 24576
TILE_F = 2048
N_TILES = FREE // TILE_F       # 12
# tiles handed to gpsimd instead of the vector engine (load balance)
GPSIMD_TILES = 0


def _build(mids, deltas, c0):
    """Build the per-core SPMD Bass program. mids/deltas are python float
    lists of length L-1; c0 = codebook[0]."""
    from contextlib import ExitStack

    import concourse.bass as bass
    import concourse.tile as tile
    from concourse import bacc, mybir

    f32 = mybir.dt.float32
    i32 = mybir.dt.int32
    Alu = mybir.AluOpType

    nc = bacc.Bacc(
        "TRN2",
        target_bir_lowering=False,
        debug=False,
        num_devices=N_CORES,
    )
    # row p = [x row | means row], so one DMA (one wait sem) per tile feeds
    # both operands with the partition dim leading in both APs.
    xm = nc.dram_tensor("xm", [P, 2 * FREE], f32, kind="ExternalInput")
    xm_r = xm.rearrange("p (h q) -> p h q", h=2)
    # per-partition replicated constants: column i holds -mids[i]
    nmid = nc.dram_tensor("nmid", [P, L], f32, kind="ExternalInput")
    sym_out = nc.dram_tensor("sym", [P, FREE], i32, kind="ExternalOutput")
    y_out = nc.dram_tensor("y", [P, FREE], f32, kind="ExternalOutput")

    Act = mybir.ActivationFunctionType
    # y_hat = sum_i (delta_i/2) * sign(r - m_i) + (c0 + sum_i delta_i/2) + means
    ybias = float(c0 + sum(d * 0.5 for d in deltas))

    with tile.TileContext(nc) as tc, ExitStack() as ctx:
        inp = ctx.enter_context(tc.tile_pool(name="inp", bufs=2))
        work = ctx.enter_context(tc.tile_pool(name="work", bufs=1))
        sgn = ctx.enter_context(tc.tile_pool(name="sgn", bufs=4))
        outp = ctx.enter_context(tc.tile_pool(name="outp", bufs=2))
        cst = ctx.enter_context(tc.tile_pool(name="cst", bufs=1))

        nmt = cst.tile([P, L], f32, tag="nmt")
        nc.gpsimd.dma_start(nmt[:], nmid[:])

        for t in range(N_TILES):
            eng = nc.gpsimd if t < GPSIMD_TILES else nc.vector
            sl = bass.ts(t, TILE_F)
            # one DMA loads both x and means tile halves -> single wait sem
            txm = inp.tile([P, 2 * TILE_F], f32, tag="txm")
            nc.gpsimd.dma_start(
                txm[:].rearrange("p (h f) -> p h f", h=2), xm_r[:, :, sl]
            )
            tx = txm[:, :TILE_F]
            tm = txm[:, TILE_F:]

            r = work.tile([P, TILE_F], f32, tag=f"r{t % 2}")
            eng.tensor_sub(r[:], tx, tm)

            # values: ACT generates s_i = sign(r - m_i); DVE accumulates
            # y += s_i * (delta_i / 2) in one fused op per level.
            ya = work.tile([P, TILE_F], f32, tag=f"ya{t % 2}")
            yb = work.tile([P, TILE_F], f32, tag=f"yb{t % 2}")
            s0 = sgn.tile([P, TILE_F], f32, tag="s")
            nc.scalar.activation(s0[:], r[:], Act.Sign, bias=nmt[:, 0:1])
            eng.tensor_scalar(
                ya[:], s0[:], deltas[0] * 0.5, None, op0=Alu.mult
            )
            ycur, ynxt = ya, yb
            # symbols: sym = sum_i (r > m_i), fused compare+add chain
            sa = work.tile([P, TILE_F], f32, tag=f"sa{t % 2}")
            sb = work.tile([P, TILE_F], f32, tag=f"sb{t % 2}")
            eng.tensor_scalar(sa[:], r[:], mids[0], None, op0=Alu.is_gt)
            scur, snxt = sa, sb
            syi = outp.tile([P, TILE_F], i32, tag="syi")
            for i in range(1, L - 1):
                si = sgn.tile([P, TILE_F], f32, tag="s")
                nc.scalar.activation(si[:], r[:], Act.Sign, bias=nmt[:, i:i + 1])
                eng.scalar_tensor_tensor(
                    ynxt[:], si[:], deltas[i] * 0.5, ycur[:],
                    op0=Alu.mult, op1=Alu.add,
                )
                ycur, ynxt = ynxt, ycur
                # write the final sym level directly as int32
                sdst = syi[:] if i == L - 2 else snxt[:]
                eng.scalar_tensor_tensor(
                    sdst, r[:], mids[i], scur[:], op0=Alu.is_gt, op1=Alu.add
                )
                scur, snxt = snxt, scur

            nc.sync.dma_start(sym_out[:, sl], syi[:])

            yh = outp.tile([P, TILE_F], f32, tag="yh")
            eng.scalar_tensor_tensor(
                yh[:], tm, ybias, ycur[:], op0=Alu.add, op1=Alu.add
            )
            nc.sync.dma_start(y_out[:, sl], yh[:])

    nc.compile()
    return nc


_cache = {}


def _get_nc(codebook):
    key = codebook.tobytes()
    if key not in _cache:
        cb = codebook.astype(np.float64)
        mids = [float(v) for v in ((cb[:-1] + cb[1:]) * 0.5).astype(np.float32)]
        deltas = [float(v) for v in (cb[1:] - cb[:-1]).astype(np.float32)]
        nmid = np.zeros((P, L), np.float32)
        nmid[:, : L - 1] = -np.float32(mids)[None, :]
        nc = _build(mids, deltas, float(codebook[0]))
        _cache[key] = (nc, nmid)
    return _cache[key]


def _run(x, means, codebook, trace=False):
    from concourse.bass_utils import run_bass_kernel_spmd

    nc, nmid = _get_nc(np.asarray(codebook))

    x = np.asarray(x).reshape(N_CORES, P, FREE)
    means = np.asarray(means).reshape(N_CORES, P, FREE)
    in_maps = [
        {
            "xm": np.ascontiguousarray(np.concatenate([x[c], means[c]], axis=1)),
            "nmid": nmid,
        }
        for c in range(N_CORES)
    ]
    res = run_bass_kernel_spmd(
        nc, in_maps, core_ids=list(range(N_CORES)), trace=trace
    )
    sym = np.stack([res.results[c]["sym"] for c in range(N_CORES)])
    y = np.stack([res.results[c]["y"] for c in range(N_CORES)])
    sym = sym.reshape(B, C, H, W).astype(np.int32)
    y = y.reshape(B, C, H, W).astype(np.float32)
    return (sym, y), res


def kernel(x, means, codebook):
    (sym, y), _ = _run(x, means, codebook)
    return sym, y


# revision 14
# speedup vs baseline: 1.1657x; 1.0603x over previous
"""Trainium2 Bass kernel for nn_AdaptedEntropyModel (vq_codebook).

reference:
    r = x - means
    symbols = argmin_i |codebook[i] - r|   (ties -> left / lower index)
    y_hat   = codebook[symbols] + means

Algorithm used here (exact, including tie behavior):
    with midpoints m_i = (c_i + c_{i+1})/2, i = 0..L-2 (codebook sorted):
        symbols = sum_i [r > m_i]
        y_hat   = c_0 + sum_i [r > m_i] * (c_{i+1} - c_i) + means

Per-core the heavy loop is 63 fused scalar_tensor_tensor ops (compare +
accumulate) for symbols and 63 fused tensor_scalar (compare * delta) +
tensor_tensor add ops for the values, all on the DVE (plus a slice of
tiles offloaded to GPSIMD).

Sharding: pure data parallel. x/means are [32,192,64,64]; each of the 8
cores gets 4 consecutive batches (a contiguous 3,145,728-element chunk),
viewed as [128, 24576] on device. x and means are interleaved host-side
into one [2, 128, FREE] input so each tile is loaded by a single DMA
(one wait semaphore - the V3 ISA allows only one sync wait per compute
instruction). No communication. The tiny codebook is turned into
per-instruction immediates at build time (kernel() re-builds if it ever
sees a different codebook).
"""

import sys

import numpy as np

if "/opt/trn_rl_repo" not in sys.path:
    sys.path.insert(0, "/opt/trn_rl_repo")

B, C, H, W = 32, 192, 64, 64
L = 64
N_CORES = 8
TOT = B * C * H * W            # 25_165_824
PER_CORE = TOT // N_CORES      # 3_145_728
P = 128
FREE = PER_CORE // P           # 24576
TILE_F = 2048
N_TILES = FREE // TILE_F       # 12
# tiles handed to gpsimd instead of the vector engine (load balance)
GPSIMD_TILES = 0


def _build(mids, deltas, c0):
    """Build the per-core SPMD Bass program. mids/deltas are python float
    lists of length L-1; c0 = codebook[0]."""
    from contextlib import ExitStack

    import concourse.bass as bass
    import concourse.tile as tile
    from concourse import bacc, mybir

    f32 = mybir.dt.float32
    i32 = mybir.dt.int32
    Alu = mybir.AluOpType

    nc = bacc.Bacc(
        "TRN2",
        target_bir_lowering=False,
        debug=False,
        num_devices=N_CORES,
    )
    # row p = [x row | means row], so one DMA (one wait sem) per tile feeds
    # both operands with the partition dim leading in both APs.
    xm = nc.dram_tensor("xm", [P, 2 * FREE], f32, kind="ExternalInput")
    xm_r = xm.rearrange("p (h q) -> p h q", h=2)
    # per-partition replicated constants: column i holds -mids[i]
    nmid = nc.dram_tensor("nmid", [P, L], f32, kind="ExternalInput")
    sym_out = nc.dram_tensor("sym", [P, FREE], i32, kind="ExternalOutput")
    y_out = nc.dram_tensor("y", [P, FREE], f32, kind="ExternalOutput")

    Act = mybir.ActivationFunctionType
    # y_hat = sum_i (delta_i/2) * sign(r - m_i) + (c0 + sum_i delta_i/2) + means
    ybias = float(c0 + sum(d * 0.5 for d in deltas))

    with tile.TileContext(nc) as tc, ExitStack() as ctx:
        inp = ctx.enter_context(tc.tile_pool(name="inp", bufs=2))
        work = ctx.enter_context(tc.tile_pool(name="work", bufs=1))
        sgn = ctx.enter_context(tc.tile_pool(name="sgn", bufs=4))
        outp = ctx.enter_context(tc.tile_pool(name="outp", bufs=2))
        cst = ctx.enter_context(tc.tile_pool(name="cst", bufs=1))

        nmt = cst.tile([P, L], f32, tag="nmt")
        nc.sync.dma_start(nmt[:], nmid[:])

        for t in range(N_TILES):
            eng = nc.gpsimd if t < GPSIMD_TILES else nc.vector
            sl = bass.ts(t, TILE_F)
            # one DMA loads both x and means tile halves -> single wait sem
            txm = inp.tile([P, 2 * TILE_F], f32, tag="txm")
            nc.sync.dma_start(
                txm[:].rearrange("p (h f) -> p h f", h=2), xm_r[:, :, sl]
            )
            tx = txm[:, :TILE_F]
            tm = txm[:, TILE_F:]

            r = work.tile([P, TILE_F], f32, tag=f"r{t % 2}")
            eng.tensor_sub(r[:], tx, tm)

            # values: ACT generates s_i = sign(r - m_i); DVE accumulates
            # y += s_i * (delta_i / 2) in one fused op per level.
            ya = work.tile([P, TILE_F], f32, tag=f"ya{t % 2}")
            yb = work.tile([P, TILE_F], f32, tag=f"yb{t % 2}")
            s0 = sgn.tile([P, TILE_F], f32, tag="s")
            nc.scalar.activation(s0[:], r[:], Act.Sign, bias=nmt[:, 0:1])
            eng.tensor_scalar(
                ya[:], s0[:], deltas[0] * 0.5, None, op0=Alu.mult
            )
            ycur, ynxt = ya, yb
            # symbols: sym = sum_i (r > m_i), fused compare+add chain
            sa = work.tile([P, TILE_F], f32, tag=f"sa{t % 2}")
            sb = work.tile([P, TILE_F], f32, tag=f"sb{t % 2}")
            eng.tensor_scalar(sa[:], r[:], mids[0], None, op0=Alu.is_gt)
            scur, snxt = sa, sb
            syi = outp.tile([P, TILE_F], i32, tag="syi")
            for i in range(1, L - 1):
                si = sgn.tile([P, TILE_F], f32, tag="s")
                nc.scalar.activation(si[:], r[:], Act.Sign, bias=nmt[:, i:i + 1])
                eng.scalar_tensor_tensor(
                    ynxt[:], si[:], deltas[i] * 0.5, ycur[:],
                    op0=Alu.mult, op1=Alu.add,
                )
                ycur, ynxt = ynxt, ycur
                # write the final sym level directly as int32
                sdst = syi[:] if i == L - 2 else snxt[:]
                eng.scalar_tensor_tensor(
                    sdst, r[:], mids[i], scur[:], op0=Alu.is_gt, op1=Alu.add
                )
                scur, snxt = snxt, scur

            nc.sync.dma_start(sym_out[:, sl], syi[:])

            yh = outp.tile([P, TILE_F], f32, tag="yh")
            eng.scalar_tensor_tensor(
                yh[:], tm, ybias, ycur[:], op0=Alu.add, op1=Alu.add
            )
            nc.sync.dma_start(y_out[:, sl], yh[:])

    nc.compile()
    return nc


_cache = {}


def _get_nc(codebook):
    key = codebook.tobytes()
    if key not in _cache:
        cb = codebook.astype(np.float64)
        mids = [float(v) for v in ((cb[:-1] + cb[1:]) * 0.5).astype(np.float32)]
        deltas = [float(v) for v in (cb[1:] - cb[:-1]).astype(np.float32)]
        nmid = np.zeros((P, L), np.float32)
        nmid[:, : L - 1] = -np.float32(mids)[None, :]
        nc = _build(mids, deltas, float(codebook[0]))
        _cache[key] = (nc, nmid)
    return _cache[key]


def _run(x, means, codebook, trace=False):
    from concourse.bass_utils import run_bass_kernel_spmd

    nc, nmid = _get_nc(np.asarray(codebook))

    x = np.asarray(x).reshape(N_CORES, P, FREE)
    means = np.asarray(means).reshape(N_CORES, P, FREE)
    in_maps = [
        {
            "xm": np.ascontiguousarray(np.concatenate([x[c], means[c]], axis=1)),
            "nmid": nmid,
        }
        for c in range(N_CORES)
    ]
    res = run_bass_kernel_spmd(
        nc, in_maps, core_ids=list(range(N_CORES)), trace=trace
    )
    sym = np.stack([res.results[c]["sym"] for c in range(N_CORES)])
    y = np.stack([res.results[c]["y"] for c in range(N_CORES)])
    sym = sym.reshape(B, C, H, W).astype(np.int32)
    y = y.reshape(B, C, H, W).astype(np.float32)
    return (sym, y), res


def kernel(x, means, codebook):
    (sym, y), _ = _run(x, means, codebook)
    return sym, y


# revision 17
# speedup vs baseline: 1.4110x; 1.2104x over previous
"""Trainium2 Bass kernel for nn_AdaptedEntropyModel (vq_codebook).

reference:
    r = x - means
    symbols = argmin_i |codebook[i] - r|   (ties -> left / lower index)
    y_hat   = codebook[symbols] + means

Algorithm (exact up to f32 boundary rounding):
  with sorted codebook c_i, midpoints m_i = (c_i + c_{i+1})/2 and
  deltas D_i = c_{i+1} - c_i (i = 0..62):
      b_i     = [r > m_i]
      symbols = sum_i b_i
      y_hat   = c_0 + sum_i D_i b_i + means

Both sums are packed into ONE fused accumulator per element:
      z = sum_i W_i * s_i,   W_i = (D_i + K)/2,   s_i = sign(r - m_i)
  so  z + C = K*symbols + y_off   (C = sum_i W_i, y_off = sum_i D_i b_i,
                                   0 <= y_off << K = 128)
      symbols = round((z + C)/K)        (f32->i32 convert rounds nearest)
      y_hat   = (z + C - K*symbols) + c_0 + means

The signs are produced on the otherwise-idle scalar engine (ACT) via
sign(fma(r, 3, beta_i)); beta_i ~ -3*m_i is nudged so its f32 mantissa is
not divisible by 3, which makes 3*r + beta_i != 0 for EVERY f32 r - the
hardware affine is a true fused multiply-add, so sign() can never return
0 and each element lands cleanly on one side (verified on silicon). The
DVE then needs just ONE fused scalar_tensor_tensor (mult, add) per level
instead of separate symbol/value chains - it is the critical path at
~2.1 us per [128 x 2048] level.

Sharding: pure data parallel over batch; each of the 8 cores gets 4
consecutive batches (contiguous 3,145,728 f32), viewed as [128, 24576].
x and means are interleaved host-side into one [128, 2*FREE] input so
each tile is loaded by a single DMA (single wait semaphore - the V3 ISA
allows only one sync wait per instruction). The codebook-derived
constants are baked per build; kernel() re-builds if the codebook
changes.
"""

import sys

import numpy as np

if "/opt/trn_rl_repo" not in sys.path:
    sys.path.insert(0, "/opt/trn_rl_repo")

B, C, H, W = 32, 192, 64, 64
L = 64
N_CORES = 8
TOT = B * C * H * W            # 25_165_824
PER_CORE = TOT // N_CORES      # 3_145_728
P = 128
FREE = PER_CORE // P           # 24576
TILE_F = 2048
N_TILES = FREE // TILE_F       # 12
K_ENC = 128.0                  # symbol step in the packed accumulator


def _coprime3_beta(m):
    """f32 beta ~ -3*m whose integer mantissa is not divisible by 3, so
    fma(r, 3, beta) is never exactly 0 for any f32 r."""
    b = np.float32(-3.0 * m)
    if b == 0.0 or not np.isfinite(b):
        b = np.float32(1e-30)
    for _ in range(4):
        mant = int(np.abs(b).view(np.uint32) & 0x7FFFFF) | 0x800000
        if mant % 3 != 0:
            return float(b)
        b = np.nextafter(b, np.float32(np.sign(b) * np.float32(1e38)),
                         dtype=np.float32)
    return float(b)


def _build(weights, betas, dec_scale, dec_bias, y_bias):
    """Build the per-core SPMD Bass program.

    weights[i] = (D_i + K)/2 (stt scalar per level)
    betas[i]   = ACT bias for level i (threshold -beta/3)
    dec_scale  = 1/K, dec_bias = C/K      (symbol decode ts)
    y_bias     = C + c_0                  (value decode stt)
    """
    from contextlib import ExitStack

    import concourse.bass as bass
    import concourse.tile as tile
    from concourse import bacc, mybir

    f32 = mybir.dt.float32
    i32 = mybir.dt.int32
    Alu = mybir.AluOpType
    Act = mybir.ActivationFunctionType

    nc = bacc.Bacc(
        "TRN2",
        target_bir_lowering=False,
        debug=False,
        num_devices=N_CORES,
    )
    # row p = [x row | means row]: one DMA per tile feeds both halves
    xm = nc.dram_tensor("xm", [P, 2 * FREE], f32, kind="ExternalInput")
    xm_r = xm.rearrange("p (h q) -> p h q", h=2)
    # per-partition replicated constants: column i holds betas[i]
    nmid = nc.dram_tensor("nmid", [P, L], f32, kind="ExternalInput")
    sym_out = nc.dram_tensor("sym", [P, FREE], i32, kind="ExternalOutput")
    y_out = nc.dram_tensor("y", [P, FREE], f32, kind="ExternalOutput")

    with tile.TileContext(nc) as tc, ExitStack() as ctx:
        inp = ctx.enter_context(tc.tile_pool(name="inp", bufs=2))
        work = ctx.enter_context(tc.tile_pool(name="work", bufs=1))
        sgn = ctx.enter_context(tc.tile_pool(name="sgn", bufs=4))
        outp = ctx.enter_context(tc.tile_pool(name="outp", bufs=2))
        cst = ctx.enter_context(tc.tile_pool(name="cst", bufs=1))

        nmt = cst.tile([P, L], f32, tag="nmt")
        nc.sync.dma_start(nmt[:], nmid[:])

        for t in range(N_TILES):
            sl = bass.ts(t, TILE_F)
            txm = inp.tile([P, 2 * TILE_F], f32, tag="txm")
            nc.sync.dma_start(
                txm[:].rearrange("p (h f) -> p h f", h=2), xm_r[:, :, sl]
            )
            tx = txm[:, :TILE_F]
            tm = txm[:, TILE_F:]

            r = work.tile([P, TILE_F], f32, tag=f"r{t % 2}")
            nc.vector.tensor_sub(r[:], tx, tm)

            # packed accumulator: z += W_i * sign(3r + beta_i)
            za = work.tile([P, TILE_F], f32, tag=f"za{t % 2}")
            zb = work.tile([P, TILE_F], f32, tag=f"zb{t % 2}")
            s0 = sgn.tile([P, TILE_F], f32, tag="s")
            nc.scalar.activation(s0[:], r[:], Act.Sign,
                                 bias=nmt[:, 0:1], scale=3.0)
            nc.vector.tensor_scalar(za[:], s0[:], weights[0], None,
                                    op0=Alu.mult)
            cur, nxt = za, zb
            for i in range(1, L - 1):
                si = sgn.tile([P, TILE_F], f32, tag="s")
                nc.scalar.activation(si[:], r[:], Act.Sign,
                                     bias=nmt[:, i:i + 1], scale=3.0)
                nc.vector.scalar_tensor_tensor(
                    nxt[:], si[:], weights[i], cur[:],
                    op0=Alu.mult, op1=Alu.add,
                )
                cur, nxt = nxt, cur

            # decode: sym = round(z/K + C/K)  (convert rounds to nearest)
            syi = outp.tile([P, TILE_F], i32, tag="syi")
            nc.vector.tensor_scalar(syi[:], cur[:], dec_scale, dec_bias,
                                    op0=Alu.mult, op1=Alu.add)
            nc.sync.dma_start(sym_out[:, sl], syi[:])

            # y_hat = (z - K*symf) + (C + c0) + means
            sf = work.tile([P, TILE_F], f32, tag=f"sf{t % 2}")
            nc.vector.tensor_scalar(sf[:], syi[:], 1.0, None, op0=Alu.mult)
            w = work.tile([P, TILE_F], f32, tag=f"w{t % 2}")
            nc.vector.scalar_tensor_tensor(
                w[:], sf[:], -K_ENC, cur[:], op0=Alu.mult, op1=Alu.add
            )
            yh = outp.tile([P, TILE_F], f32, tag="yh")
            nc.vector.scalar_tensor_tensor(
                yh[:], tm, y_bias, w[:], op0=Alu.add, op1=Alu.add
            )
            nc.sync.dma_start(y_out[:, sl], yh[:])

    nc.compile()
    return nc


_cache = {}


def _get_nc(codebook):
    key = codebook.tobytes()
    if key not in _cache:
        cb = codebook.astype(np.float64)
        mids = ((cb[:-1] + cb[1:]) * 0.5).astype(np.float32).astype(np.float64)
        deltas = (cb[1:] - cb[:-1]).astype(np.float64)
        weights = [float(np.float32((d + K_ENC) * 0.5)) for d in deltas]
        betas = [_coprime3_beta(m) for m in mids]
        const = float(sum(np.float64(w) for w in weights))
        dec_scale = float(np.float32(1.0 / K_ENC))
        dec_bias = float(np.float32(const / K_ENC))
        y_bias = float(np.float32(const + cb[0]))
        nmid = np.zeros((P, L), np.float32)
        nmid[:, : L - 1] = np.float32(betas)[None, :]
        nc = _build(weights, betas, dec_scale, dec_bias, y_bias)
        _cache[key] = (nc, nmid)
    return _cache[key]


def _run(x, means, codebook, trace=False):
    from concourse.bass_utils import run_bass_kernel_spmd

    nc, nmid = _get_nc(np.asarray(codebook))

    x = np.asarray(x).reshape(N_CORES, P, FREE)
    means = np.asarray(means).reshape(N_CORES, P, FREE)
    in_maps = [
        {
            "xm": np.ascontiguousarray(np.concatenate([x[c], means[c]], axis=1)),
            "nmid": nmid,
        }
        for c in range(N_CORES)
    ]
    res = run_bass_kernel_spmd(
        nc, in_maps, core_ids=list(range(N_CORES)), trace=trace
    )
    sym = np.stack([res.results[c]["sym"] for c in range(N_CORES)])
    y = np.stack([res.results[c]["y"] for c in range(N_CORES)])
    sym = sym.reshape(B, C, H, W).astype(np.int32)
    y = y.reshape(B, C, H, W).astype(np.float32)
    return (sym, y), res


def kernel(x, means, codebook):
    (sym, y), _ = _run(x, means, codebook)
    return sym, y
